# revision 3
# baseline (speedup 1.0000x reference)
"""Bilateral filter (d=7, sigma_color=0.1, sigma_space=3.0) on 8 Trainium2 cores.

Input x: [16, 3, 768, 768] fp32.  out = sum_{(i,j)!=0, |i|,|j|<=7} sw[i,j] *
exp(-50*(s_ij - x)^2) * s_ij  with s_ij the reflect-padded shifted window.

Strategy (per core = 2 images x 3 channels = 6 planes, data-parallel):
- Partitions carry (plane, row-strip): 6 planes x 21 strips of 37 rows = 126
  partitions. Both spatial dims live in the free dimension so the (i,j)
  window shifts are plain strided AP reads (lanes cannot shift partitions).
- Host reflect-pads each plane to [791, 782] (768+14 cols, 768+14+9 rows;
  the 9 extra bottom rows feed only discarded strip-tail outputs).
- Per 96-column chunk (8 chunks): load slab [126, 51, 110], then per offset:
    diff = win - center         (DVE / GPSIMD, alternating offsets)
    g    = Derivative_Erf(sqrt(50)*diff)   (ACT; = 2/sqrt(pi)*exp(-50 diff^2))
    t    = (g * (sw_ij*sqrt(pi)/2)) * win  (fused scalar_tensor_tensor, DVE)
    psum += I @ t               (TensorE identity-matmul accumulate, 7 banks)
  Evacuate PSUM via ACT copy, DMA out.
"""
import numpy as np

D = 7
SIGMA_COLOR = 0.1
SIGMA_SPACE = 3.0

N_CORES = 8
PLANES = 6            # per-core planes (2 images x 3 channels)
STRIPS = 21           # row-strips per plane
SH = 37               # strip height -> 21*37 = 777 >= 768
P_USED = PLANES * STRIPS   # 126 partitions
H = W = 768
HP = STRIPS * SH      # 777 padded output rows per plane
XROWS = SH * (STRIPS - 1) + SH + 2 * D  # 791 input rows needed per plane
XCOLS = W + 2 * D     # 782
WC = 96               # column chunk width
NCHUNK = W // WC      # 8
CHUNK_F = SH * WC     # 3552 output elems per partition per chunk
NSLICE = (CHUNK_F + 511) // 512   # 7 PSUM bank slices

_CACHE = {}


def _sw_table():
    offs = np.arange(-D, D + 1)
    sw = np.exp(-0.5 * (offs[:, None] ** 2 + offs[None, :] ** 2) / SIGMA_SPACE ** 2)
    return (sw / sw.sum()).astype(np.float32)


def build(reps=1):
    import concourse.tile as tile
    import concourse.bass as bass
    from concourse import bacc, mybir
    from concourse.masks import make_identity

    f32 = mybir.dt.float32
    nc = bacc.Bacc("TRN2", target_bir_lowering=False, debug=False,
                   num_devices=N_CORES)
    xp = nc.dram_tensor("xp", [PLANES, XROWS, XCOLS], f32, kind="ExternalInput")
    out = nc.dram_tensor("out", [P_USED * SH, W], f32, kind="ExternalOutput")

    sw = _sw_table()
    CDERF = float(np.sqrt(np.pi) / 2.0)
    SCALE = float(np.sqrt(0.5 / SIGMA_COLOR ** 2))  # sqrt(50)

    offsets = [(i, j) for i in range(-D, D + 1) for j in range(-D, D + 1)
               if not (i == 0 and j == 0)]
    NOFF = len(offsets)  # 224

    out3 = out.ap().rearrange("(p r) w -> p r w", r=SH)

    with tile.TileContext(nc) as tc:
        with (
            tc.tile_pool(name="consts", bufs=1) as consts,
            tc.tile_pool(name="slab_pool", bufs=2) as slab_pool,
            tc.tile_pool(name="diff_pool", bufs=3) as diff_pool,
            tc.tile_pool(name="g_pool", bufs=4) as g_pool,
            tc.tile_pool(name="outb_pool", bufs=2) as outb_pool,
            tc.tile_pool(name="psum_pool", bufs=1, space="PSUM") as psum_pool,
        ):
            ident = consts.tile([128, 128], f32)
            make_identity(nc, ident)
            identT = ident[0:P_USED, 0:P_USED]

            def body(_iv=None):
                for t in range(NCHUNK):
                    slab = slab_pool.tile([128, SH + 2 * D, WC + 2 * D], f32,
                                          tag="slab")
                    for c in range(PLANES):
                        src = bass.AP(
                            tensor=xp, offset=c * XROWS * XCOLS + WC * t,
                            ap=[[SH * XCOLS, STRIPS], [XCOLS, SH + 2 * D],
                                [1, WC + 2 * D]])
                        nc.sync.dma_start(out=slab[STRIPS * c:STRIPS * (c + 1)],
                                          in_=src)

                    psum = psum_pool.tile([128, NSLICE, 512], f32, tag="psum")
                    center = slab[0:P_USED, D:D + SH, D:D + WC]
                    for o, (i, j) in enumerate(offsets):
                        win = slab[0:P_USED, D + i:D + i + SH, D + j:D + j + WC]
                        diff = diff_pool.tile([128, SH, WC], f32, tag="diff")
                        eng = nc.gpsimd if (o % 2 == 0) else nc.vector
                        eng.tensor_tensor(diff[0:P_USED], win, center,
                                          mybir.AluOpType.subtract)
                        g = g_pool.tile([128, SH, WC], f32, tag="g")
                        nc.scalar.activation(
                            g[0:P_USED], diff[0:P_USED],
                            mybir.ActivationFunctionType.Derivative_Erf,
                            scale=SCALE)
                        c_o = float(sw[D + i, D + j]) * CDERF
                        nc.vector.scalar_tensor_tensor(
                            g[0:P_USED], g[0:P_USED], c_o, win,
                            mybir.AluOpType.mult, mybir.AluOpType.mult)
                        gf = g.rearrange("p a b -> p (a b)")
                        for k in range(NSLICE):
                            n0 = k * 512
                            n1 = min(CHUNK_F, n0 + 512)
                            nc.tensor.matmul(
                                psum[0:P_USED, k, 0:n1 - n0], identT,
                                gf[0:P_USED, n0:n1],
                                start=(o == 0), stop=(o == NOFF - 1))
                    outb = outb_pool.tile([128, NSLICE * 512], f32, tag="outb")
                    nc.scalar.copy(outb[0:P_USED],
                                   psum[0:P_USED].rearrange("p a b -> p (a b)"))
                    nc.sync.dma_start(
                        out=out3[:, :, WC * t:WC * t + WC],
                        in_=outb[0:P_USED, 0:CHUNK_F].rearrange(
                            "p (r c) -> p r c", c=WC))

            if reps == 1:
                body()
            else:
                with tc.For_i(0, reps, 1) as _i:
                    body(_i)
    nc.compile()
    return nc


def _prepare_inputs(x):
    """x: [16,3,768,768] fp32 -> per-core padded plane stacks [6,791,782]."""
    planes = np.ascontiguousarray(x.reshape(N_CORES, PLANES, H, W))
    in_maps = []
    for c in range(N_CORES):
        xp = np.pad(planes[c], ((0, 0), (D, D + (XROWS - H - 2 * D)), (D, D)),
                    mode="reflect")
        in_maps.append({"xp": xp})
    return in_maps


def _gather_outputs(results):
    outs = []
    for c in range(N_CORES):
        o = results[c]["out"].reshape(PLANES, HP, W)[:, :H, :]
        outs.append(o)
    return np.stack(outs).reshape(16, 3, H, W).astype(np.float32)


def kernel(x):
    from concourse.bass_utils import run_bass_kernel_spmd

    x = np.asarray(x, dtype=np.float32)
    if "nc" not in _CACHE:
        _CACHE["nc"] = build(reps=1)
    in_maps = _prepare_inputs(x)
    res = run_bass_kernel_spmd(_CACHE["nc"], in_maps,
                               core_ids=list(range(N_CORES)))
    return _gather_outputs(res.results)
